# revision 13
# baseline (speedup 1.0000x reference)
"""Trainium2 Bass kernel for single-head causal attention.

x:[4,4096,1024] f32, W_q/W_k/W_v:[1024,64], W_o:[64,1024].
Sharding: 8 cores = 4 batches x 2 query-halves. Each core computes
attention for 2048 queries against all 4096 keys of its batch.

One SPMD program; every per-core difference (which batch, which query
half, the causal mask content) is carried in the input data.
"""

import sys

for _p in ("/opt/trn_rl_repo",):
    if _p not in sys.path:
        sys.path.insert(0, _p)

import numpy as np

D_MODEL = 1024
D_HEAD = 64
SEQ = 4096
BATCH = 4
NCORES = 8
NQ = 2048          # queries per core
P = 128
DCH = D_MODEL // P  # 8 contraction chunks
KC = SEQ // P       # 32 key chunks
QG = NQ // 512      # 4 query groups of 512
QT = NQ // P        # 16 query tiles of 128
MW = 6016           # mask table width (u in [-3968, 2048))
MB = 3968           # mask table base offset

_prog = None


def _build_program():
    import concourse.bacc as bacc
    import concourse.mybir as mybir
    import concourse.tile as tile

    fp32 = mybir.dt.float32
    nc = bacc.Bacc("TRN2", target_bir_lowering=False, debug=False)

    xt = nc.dram_tensor("xt", [D_MODEL, SEQ], fp32, kind="ExternalInput")
    xtq = nc.dram_tensor("xtq", [D_MODEL, NQ], fp32, kind="ExternalInput")
    wkv = nc.dram_tensor("wkv", [D_MODEL, P], fp32, kind="ExternalInput")
    wq = nc.dram_tensor("wq", [D_MODEL, D_HEAD], fp32, kind="ExternalInput")
    wo = nc.dram_tensor("wo", [D_HEAD, D_MODEL], fp32, kind="ExternalInput")
    msk = nc.dram_tensor("msk", [P, MW], fp32, kind="ExternalInput")
    y = nc.dram_tensor("y", [NQ, D_MODEL], fp32, kind="ExternalOutput")

    with tile.TileContext(nc) as tc:
        with (
            tc.tile_pool(name="singles", bufs=1) as singles,
            tc.tile_pool(name="work", bufs=3) as work,
            tc.tile_pool(name="mm_ps", bufs=2, space="PSUM") as mm_ps,
            tc.tile_pool(name="s_ps", bufs=2, space="PSUM") as s_ps_pool,
            tc.tile_pool(name="pv_ps", bufs=1, space="PSUM") as pv_pool,
        ):
            # ---- persistent SBUF tensors ----
            wkv_sb = singles.tile([P, DCH, P], fp32, tag="wkv_sb")
            nc.sync.dma_start(
                out=wkv_sb, in_=wkv.rearrange("(c p) m -> p c m", p=P)
            )
            wq_sb = singles.tile([P, DCH, D_HEAD], fp32, tag="wq_sb")
            nc.sync.dma_start(
                out=wq_sb, in_=wq.rearrange("(c p) m -> p c m", p=P)
            )
            wo_sb = singles.tile([D_HEAD, D_MODEL], fp32, tag="wo_sb")
            nc.sync.dma_start(out=wo_sb, in_=wo[:, :])
            msk_sb = singles.tile([P, MW], fp32, tag="msk_sb")
            nc.sync.dma_start(out=msk_sb, in_=msk[:, :])
            ident = singles.tile([P, D_HEAD], fp32, tag="ident")
            from concourse.masks import make_identity

            make_identity(nc, ident[D_HEAD:P, :])
            one_sb = singles.tile([1, 1], fp32, tag="one_sb")
            nc.vector.memset(one_sb, 1.0)

            kvt = singles.tile([P, SEQ], fp32, tag="kvt")  # rows 0:64 K^T, 64:128 V^T
            qt_sb = singles.tile([D_HEAD, NQ], fp32, tag="qt_sb")  # Q^T (pre-scaled)
            vaug = singles.tile([P, KC, D_HEAD + 1], fp32, tag="vaug")  # [V|1] per chunk
            nc.vector.memset(vaug[:, :, D_HEAD : D_HEAD + 1], 1.0)
            ot = singles.tile([D_HEAD + 1, NQ], fp32, tag="ot")  # O^T + den row
            rden = singles.tile([1, NQ], fp32, tag="rden")
            rdent = singles.tile([P, QT], fp32, tag="rdent")

            # ---- Q^T projection: lhsT=wq chunk, rhs=xtq chunk ----
            for qs in range(QG):
                ps = mm_ps.tile([D_HEAD, 512], fp32, tag="proj_ps")
                for dc in range(DCH):
                    xq_t = work.tile([P, 512], fp32, tag="x_t")
                    nc.sync.dma_start(
                        out=xq_t,
                        in_=xtq[dc * P : (dc + 1) * P, qs * 512 : (qs + 1) * 512],
                    )
                    nc.tensor.matmul(
                        ps,
                        lhsT=wq_sb[:, dc, :],
                        rhs=xq_t[:, :],
                        start=(dc == 0),
                        stop=(dc == DCH - 1),
                    )
                nc.vector.tensor_copy(
                    out=qt_sb[:, qs * 512 : (qs + 1) * 512], in_=ps
                )

            # PV accumulators (one per query group), live across all key chunks
            pv = [
                pv_pool.tile([D_HEAD + 1, 512], fp32, tag=f"pv{g}", name=f"pv{g}")
                for g in range(QG)
            ]

            # ---- stream over key chunks: projection, V transpose, attention ----
            for sc in range(8):  # 512-wide chunks of the key sequence
                ps = mm_ps.tile([P, 512], fp32, tag="proj_ps")
                for dc in range(DCH):
                    x_t = work.tile([P, 512], fp32, tag="x_t")
                    nc.sync.dma_start(
                        out=x_t,
                        in_=xt[dc * P : (dc + 1) * P, sc * 512 : (sc + 1) * 512],
                    )
                    nc.tensor.matmul(
                        ps,
                        lhsT=wkv_sb[:, dc, :],
                        rhs=x_t[:, :],
                        start=(dc == 0),
                        stop=(dc == DCH - 1),
                    )
                nc.vector.tensor_copy(
                    out=kvt[:, sc * 512 : (sc + 1) * 512], in_=ps
                )

                for t in range(4):  # V^T 128-col blocks -> V natural chunks
                    kc = sc * 4 + t
                    tp = mm_ps.tile([P, D_HEAD], fp32, tag="proj_ps")
                    nc.tensor.transpose(
                        tp,
                        kvt[D_HEAD:P, kc * P : (kc + 1) * P],
                        ident[D_HEAD:P, :],
                    )
                    nc.vector.tensor_copy(out=vaug[:, kc, :D_HEAD], in_=tp)

                for t in range(4):  # attention blocks for these keys
                    kc = sc * 4 + t
                    for qg in range(QG):
                        if kc >= 20 + 4 * qg:
                            # masked even for the high-query role: dead on all cores
                            continue
                        sps = s_ps_pool.tile([P, 512], fp32, tag="s_ps")
                        nc.tensor.matmul(
                            sps,
                            lhsT=kvt[:D_HEAD, kc * P : (kc + 1) * P],
                            rhs=qt_sb[:, qg * 512 : (qg + 1) * 512],
                            start=True,
                            stop=True,
                        )
                        p_t = work.tile([P, 512], fp32, tag="p_t")
                        nc.scalar.activation(
                            p_t, sps, mybir.ActivationFunctionType.Exp
                        )
                        if kc > 4 * qg - 1:
                            off = MB + 512 * qg - P * kc
                            nc.vector.tensor_tensor(
                                p_t,
                                p_t,
                                msk_sb[:, off : off + 512],
                                mybir.AluOpType.mult,
                            )
                        nc.tensor.matmul(
                            pv[qg],
                            lhsT=vaug[:, kc, :],
                            rhs=p_t[:, :],
                            start=(kc == 0),
                            stop=(kc == 19 + 4 * qg),
                            skip_group_check=True,
                        )

            # ---- finalize: O^T, denominators, output projection ----
            for qg in range(QG):
                nc.vector.tensor_copy(
                    out=ot[:, qg * 512 : (qg + 1) * 512], in_=pv[qg]
                )
            nc.vector.reciprocal(rden, ot[D_HEAD : D_HEAD + 1, :])
            for qt in range(QT):
                tp = mm_ps.tile([P, 1], fp32, tag="proj_ps")
                nc.tensor.matmul(
                    tp,
                    lhsT=rden[:, qt * P : (qt + 1) * P],
                    rhs=one_sb,
                    start=True,
                    stop=True,
                )
                nc.vector.tensor_copy(out=rdent[:, qt : qt + 1], in_=tp)

            for qt in range(QT):
                for no in range(2):
                    yp = mm_ps.tile([P, 512], fp32, tag="proj_ps")
                    nc.tensor.matmul(
                        yp,
                        lhsT=ot[:D_HEAD, qt * P : (qt + 1) * P],
                        rhs=wo_sb[:, no * 512 : (no + 1) * 512],
                        start=True,
                        stop=True,
                    )
                    y_sb = work.tile([P, 512], fp32, tag="y_sb")
                    nc.vector.tensor_scalar_mul(
                        y_sb, yp, rdent[:, qt : qt + 1]
                    )
                    nc.sync.dma_start(
                        out=y[qt * P : (qt + 1) * P, no * 512 : (no + 1) * 512],
                        in_=y_sb,
                    )

    nc.finalize()
    return nc


def _get_program():
    global _prog
    if _prog is None:
        _prog = _build_program()
    return _prog


def _make_mask(qoff: int) -> np.ndarray:
    # b01[i, MB + u] = 1.0 iff key (i + 128*kc) <= query (qoff + 512*qg + j),
    # with u = 512*qg - 128*kc + j.  Slice per block at off = MB + 512*qg - 128*kc.
    i = np.arange(P)[:, None]
    u = np.arange(MW)[None, :] - MB
    return (qoff + u - i >= 0).astype(np.float32)


def kernel(x, W_q, W_k, W_v, W_o, _trace=False):
    from concourse.bass_utils import run_bass_kernel_spmd

    nc = _get_program()

    x = np.asarray(x, dtype=np.float32)
    wq = np.ascontiguousarray(np.asarray(W_q, dtype=np.float32)) * np.float32(
        1.0 / np.sqrt(D_HEAD)
    )
    wkv = np.ascontiguousarray(
        np.concatenate(
            [np.asarray(W_k, dtype=np.float32), np.asarray(W_v, dtype=np.float32)],
            axis=1,
        )
    )
    wo = np.ascontiguousarray(np.asarray(W_o, dtype=np.float32))

    masks = [_make_mask(0), _make_mask(NQ)]
    in_maps = []
    for c in range(NCORES):
        b, half = c // 2, c % 2
        xt = np.ascontiguousarray(x[b].T)  # [1024, 4096]
        qoff = half * NQ
        in_maps.append(
            {
                "xt": xt,
                "xtq": np.ascontiguousarray(xt[:, qoff : qoff + NQ]),
                "wkv": wkv,
                "wq": wq,
                "wo": wo,
                "msk": masks[half],
            }
        )

    res = run_bass_kernel_spmd(nc, in_maps, core_ids=list(range(NCORES)))
    out = np.empty((BATCH, SEQ, D_MODEL), dtype=np.float32)
    for c in range(NCORES):
        b, half = c // 2, c % 2
        out[b, half * NQ : (half + 1) * NQ, :] = res.results[c]["y"]
    return out
